# revision 62
# baseline (speedup 1.0000x reference)
import sys

if "/opt/trn_rl_repo" not in sys.path:
    sys.path.insert(0, "/opt/trn_rl_repo")

import numpy as np

B, T, C = 2, 2048, 2048
H, H_KV = 16, 8
D = C // H  # 128
NCORES = 8
HL = H // NCORES  # 2 local query heads per core; 1 kv head per core

F32R_SCALE = 0.08838834764831845  # 1/sqrt(128)


def build_nc(b=B, t=T, c=C, mmdt="bf16"):
    """Build the per-core Bass program. Same program on all 8 cores; the
    sharding lives entirely in the input data each core receives."""
    import concourse.bass as bass  # noqa: F401
    import concourse.mybir as mybir
    import concourse.tile as tile
    from concourse import bacc

    f32 = mybir.dt.float32
    f32r = mybir.dt.float32r if mmdt == "f32r" else mybir.dt.bfloat16
    EXP = mybir.ActivationFunctionType.Exp

    ncb = c // 128  # contraction blocks for projections
    nt = t // 512  # 512-wide t tiles
    njb_per_t = 512 // 128  # 4 k-blocks per 512 q-tile

    nc = bacc.Bacc("TRN2", target_bir_lowering=False, debug=False)

    xT = nc.dram_tensor("xT", [b, c, t], f32r, kind="ExternalInput")
    wq = nc.dram_tensor("wq", [c, HL * D], f32r, kind="ExternalInput")
    wk = nc.dram_tensor("wk", [c, D], f32r, kind="ExternalInput")
    wv = nc.dram_tensor("wv", [c, D], f32r, kind="ExternalInput")
    wp = nc.dram_tensor("wp", [HL * D, c], f32r, kind="ExternalInput")
    cos2 = nc.dram_tensor("cos2", [128, t], f32, kind="ExternalInput")
    sin2 = nc.dram_tensor("sin2", [128, t], f32, kind="ExternalInput")
    maskf = nc.dram_tensor("maskf", [128, 512], f32r, kind="ExternalInput")
    onesv = nc.dram_tensor("onesv", [128, 128], f32r, kind="ExternalInput")
    ident = nc.dram_tensor("ident", [128, 128], f32r, kind="ExternalInput")
    y = nc.dram_tensor("y", [b, t, c], f32, kind="ExternalOutput")

    with tile.TileContext(nc) as tc:
        with (
            tc.tile_pool(name="wts", bufs=1) as wpool,
            tc.tile_pool(name="data", bufs=1) as dpool,
            tc.tile_pool(name="work", bufs=2) as wkp,
            tc.tile_pool(name="psum", bufs=1, space="PSUM") as pp,
        ):
            # ---- resident weights / tables ----
            # weight chunks: all single-cb, self-paced inside the first i4
            # loop so the weight stream never collides with the xt stream
            chunks = [(cb, cb + 1) for cb in range(ncb)]
            cb2chunk = {}
            for wi, (lo, hi) in enumerate(chunks):
                for cb in range(lo, hi):
                    cb2chunk[cb] = (wi, cb - lo)
            wq_sbs, wk_sbs, wv_sbs = [], [], []
            xt_pre = {}

            def weight_chunk(wi):
                lo, hi = chunks[wi]
                nwc = hi - lo
                cbs = slice(lo * 128, hi * 128)
                wq_i = wpool.tile([128, nwc * HL * D], f32r, name=f"wq{wi}")
                nc.scalar.dma_start(
                    wq_i[:].rearrange("p (cb d) -> p cb d", d=HL * D),
                    wq[cbs, :].rearrange("(cb p) d -> p cb d", p=128),
                )
                wq_sbs.append(wq_i)
                wk_i = wpool.tile([128, nwc * D], f32r, name=f"wk{wi}")
                nc.scalar.dma_start(
                    wk_i[:].rearrange("p (cb d) -> p cb d", d=D),
                    wk[cbs, :].rearrange("(cb p) d -> p cb d", p=128),
                )
                wk_sbs.append(wk_i)
                wv_i = wpool.tile([128, nwc * D], f32r, name=f"wv{wi}")
                nc.scalar.dma_start(
                    wv_i[:].rearrange("p (cb d) -> p cb d", d=D),
                    wv[cbs, :].rearrange("(cb p) d -> p cb d", p=128),
                )
                wv_sbs.append(wv_i)

            # first two weight cbs up front (scalar), xt prefetch on sync;
            # remaining weight cbs are emitted inside the first i4's loop,
            # two cbs ahead of their consumption
            weight_chunk(0)
            weight_chunk(1)
            for cb in range(8):
                xtp = wkp.tile([128, 512], f32r, tag="xt", bufs=12, name=f"xtp{cb}")
                nc.sync.dma_start(xtp[:], xT[0, cb * 128 : (cb + 1) * 128, 0:512])
                xt_pre[(0, 0, cb)] = xtp
            # big tables are DMAed lazily inside the proj phase (scalar-queue
            # slack) so they don't delay the odd-cb xt stream; allocate here
            cos_sb = wpool.tile([128, t], f32)
            sin_sb = wpool.tile([128, t], f32)
            mask_sb = wpool.tile([128, 512], f32r)
            ones_sb = wpool.tile([128, 128], f32r)
            id_sb = wpool.tile([128, 128], f32r)
            wp_sb = wpool.tile([128, HL * c], f32r)  # [p, (f, cout)]

            def load_tables(stage):
                if stage == 0:
                    nc.scalar.dma_start(cos_sb[:], cos2[:, :])
                    nc.scalar.dma_start(sin_sb[:], sin2[:, :])
                elif stage == 1:
                    nc.scalar.dma_start(mask_sb[:], maskf[:, :])
                    nc.scalar.dma_start(ones_sb[:], onesv[:, :])
                    nc.scalar.dma_start(id_sb[:], ident[:, :])
                    warm = wpool.tile([128, 1], f32)
                    nc.scalar.activation(warm[:], cos_sb[:, 0:1], EXP, scale=1.0)
                elif stage == 2:
                    nc.scalar.dma_start(
                        wp_sb[:].rearrange("p (f n) -> p f n", n=c),
                        wp.rearrange("(f p) n -> p f n", p=128),
                    )

            # ---- per-batch persistent tiles (both batches resident) ----
            QT = [
                [dpool.tile([128, t], f32r, name=f"QT{bi}_{h}") for h in range(HL)]
                for bi in range(b)
            ]
            KT = [dpool.tile([128, t], f32r, name=f"KT{bi}") for bi in range(b)]
            VT = [dpool.tile([128, t], f32r, name=f"VT{bi}") for bi in range(b)]
            Vn = [dpool.tile([128, t], f32r, name=f"Vn{bi}") for bi in range(b)]
            AT = [
                [dpool.tile([128, t], f32r, name=f"AT{bi}_{h}") for h in range(HL)]
                for bi in range(b)
            ]

            swap_mask = [i ^ 1 for i in range(32)]

            # ---- phase 1: QKV projections (+ fused RoPE) for both batches ----
            for bi in range(b):
                for i4 in range(nt):
                    ts_ = slice(i4 * 512, (i4 + 1) * 512)
                    ps = {
                        kind: pp.tile([128, 512], f32, tag="proj", bufs=3, name=f"ps_{kind}")
                        for kind in ("q0", "q1", "k")
                    }
                    ps["v"] = pp.tile([128, 512], f32, tag="av", bufs=2, name="ps_v")
                    xts = {}

                    def get_xt(cb):
                        if (bi, i4, cb) in xt_pre:
                            xt = xt_pre.pop((bi, i4, cb))
                        else:
                            xt = wkp.tile([128, 512], f32r, tag="xt", bufs=12)
                            # alternate DMA queues: scalar is idle during the
                            # projection phase, halving per-queue serialization
                            eng = nc.sync if cb % 2 == 0 else nc.scalar
                            eng.dma_start(xt[:], xT[bi, cb * 128 : (cb + 1) * 128, ts_])
                        return xt

                    def mm(kind, cb):
                        st, sp = (cb == 0), (cb == ncb - 1)
                        wi, cbl = cb2chunk[cb]
                        base = cbl * HL * D
                        w = {
                            "q0": lambda: wq_sbs[wi][:, base : base + 128],
                            "q1": lambda: wq_sbs[wi][:, base + 128 : base + 256],
                            "k": lambda: wk_sbs[wi][:, cbl * 128 : (cbl + 1) * 128],
                            "v": lambda: wv_sbs[wi][:, cbl * 128 : (cbl + 1) * 128],
                        }[kind]()
                        nc.tensor.matmul(ps[kind][:], w, xts[cb][:], start=st, stop=sp)

                    # cb-major for the bulk; kind-major for the last two cb so
                    # q0/q1/k stop early and rope can free their psums before
                    # the next i4's first matmuls need them
                    for cb in range(ncb - 2):
                        xts[cb] = get_xt(cb)
                        if bi == 0 and i4 == 0 and cb + 2 < ncb:
                            weight_chunk(cb + 2)
                        for kind in ("q0", "q1", "k", "v"):
                            mm(kind, cb)
                    for cb in (ncb - 2, ncb - 1):
                        xts[cb] = get_xt(cb)
                    # lazy table loads ride the scalar queue's slack after
                    # this i4's odd-cb xt loads have been issued
                    if bi == 0 and i4 < 3:
                        load_tables(i4)
                    for kind in ("q0", "q1", "k", "v"):
                        mm(kind, ncb - 2)
                        mm(kind, ncb - 1)
                    # RoPE in pair-interleaved head layout (host permuted Wq/Wk
                    # columns so rotate-half pairs are adjacent partitions):
                    # dest = psum*cosI + swap_adjacent(psum)*sinS
                    def rope():
                        # pass 1 reads the psums (frees the proj ring for the
                        # next i4); pass 2 finishes from SBUF copies
                        kinds = (
                            ("q0", QT[bi][0]),
                            ("q1", QT[bi][1]),
                            ("k", KT[bi]),
                        )
                        rab = {}
                        for kind, _ in kinds:
                            ra = wkp.tile([128, 512], f32, tag="ra", bufs=3)
                            rb = wkp.tile([128, 512], f32, tag="rb", bufs=3)
                            nc.vector.tensor_mul(ra[:], ps[kind][:], cos_sb[:, ts_])
                            nc.vector.stream_shuffle(rb[:], ps[kind][:], swap_mask)
                            rab[kind] = (ra, rb)
                        for kind, dest in kinds:
                            ra, rb = rab[kind]
                            nc.vector.tensor_mul(rb[:], rb[:], sin_sb[:, ts_])
                            nc.vector.tensor_add(dest[:, ts_], ra[:], rb[:])

                    def transposes(jlo, jhi, alt=False):
                        # V natural layout ([t,d] blocks) via PE transpose.
                        # alt=True (only safe once the v accumulator is copied
                        # out): first two on "s" then the rest on "av", so the
                        # scores ring is released first for the attention phase
                        for idx, j in enumerate(range(max(jlo, 0), jhi)):
                            js = slice(j * 128, (j + 1) * 128)
                            tg = "av" if (alt and idx >= 2) else "s"
                            pt = pp.tile([128, 128], f32r, tag=tg, bufs=2)
                            nc.tensor.transpose(pt[:], VT[bi][:, js], id_sb[:])
                            nc.vector.tensor_copy(Vn[bi][:, js], pt[:])

                    if i4 == nt - 1 and bi == b - 1:
                        # attention follows immediately: drain V transposes
                        # before rope so their psum frees fast
                        nc.vector.tensor_copy(VT[bi][:, ts_], ps["v"][:])
                        transposes((i4 - 1) * 4, (i4 + 1) * 4, alt=True)
                        rope()
                    elif i4 == nt - 1:
                        rope()
                        nc.vector.tensor_copy(VT[bi][:, ts_], ps["v"][:])
                        transposes((i4 - 1) * 4, (i4 + 1) * 4)
                    else:
                        # rope first: its psum reads release the proj ring for
                        # the next i4; transposes deferred one i4 so they never
                        # wait on the VT copy
                        rope()
                        nc.vector.tensor_copy(VT[bi][:, ts_], ps["v"][:])
                        transposes((i4 - 1) * 4, i4 * 4)

            # ---- phase 2: attention (+ interleaved output projection) ----
            def po_tile(obi, it, n, last):
                po = pp.tile([128, 512], f32, tag="proj", bufs=3)
                for hh in range(HL):
                    nc.tensor.matmul(
                        po[:],
                        AT[obi][hh][:, it * 128 : (it + 1) * 128],
                        wp_sb[:, hh * c + n * 512 : hh * c + (n + 1) * 512],
                        start=(hh == 0), stop=(hh == HL - 1),
                        skip_group_check=True,
                    )
                par = (it * (c // 512) + n) % 2
                po_sb = wkp.tile([128, 512], f32, tag="yout", bufs=4)
                if last and par == 1:
                    nc.scalar.copy(po_sb[:], po[:])
                    dmae = nc.scalar
                else:
                    nc.vector.tensor_copy(po_sb[:], po[:])
                    dmae = nc.sync
                dmae.dma_start(
                    y[obi, it * 128 : (it + 1) * 128, n * 512 : (n + 1) * 512],
                    po_sb[:],
                )

            def outproj_tiles(obi, oi4, last):
                return [
                    (lambda obi=obi, it=it, n=n: po_tile(obi, it, n, last))
                    for it in range(oi4 * 4, (oi4 + 1) * 4)
                    for n in range(c // 512)
                ]

            for bi in range(b):
                for i4 in range(nt):
                    # outproj work pending from the previous i4 (or previous
                    # batch), interleaved into this i4's attention j-loops to
                    # fill exp-latency stalls on the PE
                    if i4 > 0:
                        pending = outproj_tiles(bi, i4 - 1, last=False)
                    elif bi > 0:
                        pending = outproj_tiles(bi - 1, nt - 1, last=False)
                    else:
                        pending = []
                    nslots = 2 * (njb_per_t * (i4 + 1))  # j iters this i4
                    per_slot = -(-len(pending) // nslots) if pending else 0

                    for h in range(HL):
                        qs = slice(i4 * 512, (i4 + 1) * 512)
                        pav = pp.tile([128, 512], f32, tag="av", bufs=2)
                        pden = pp.tile([128, 512], f32, tag="den", bufs=1)
                        jmax = njb_per_t * (i4 + 1) - 1
                        E_tiles = {}

                        def denav(j):
                            diag = j - njb_per_t * i4
                            off = max(diag, 0) * 128
                            Ej = E_tiles.pop(j)
                            nc.tensor.matmul(
                                pden[:, off:512],
                                ones_sb[:, 0:128],
                                Ej[:, off:512],
                                start=(j == 0), stop=(j == jmax),
                                skip_group_check=True,
                            )
                            nc.tensor.matmul(
                                pav[:, off:512],
                                Vn[bi][:, j * 128 : (j + 1) * 128],
                                Ej[:, off:512],
                                start=(j == 0), stop=(j == jmax),
                                skip_group_check=True,
                            )

                        depth = 2  # scores run this many j ahead of den/av
                        for j in range(jmax + 1):
                            diag = j - njb_per_t * i4
                            off = max(diag, 0) * 128  # skip q cols left of diag
                            pst = pp.tile([128, 512], f32, tag="s", bufs=2)
                            nc.tensor.matmul(
                                pst[:, off:512],
                                KT[bi][:, j * 128 : (j + 1) * 128],
                                QT[bi][h][:, i4 * 512 + off : (i4 + 1) * 512],
                                start=True, stop=True,
                            )
                            E = wkp.tile([128, 512], f32r, tag="E", bufs=6)
                            nc.scalar.activation(
                                E[:, off:512], pst[:, off:512], EXP, scale=F32R_SCALE
                            )
                            if diag >= 0:
                                # zero strictly-lower triangle of the diag block
                                nc.vector.tensor_mul(
                                    E[:, off : off + 128],
                                    E[:, off : off + 128],
                                    mask_sb[:, 384:512],
                                )
                            E_tiles[j] = E
                            if j >= depth:
                                denav(j - depth)
                            if j >= 1 or h == 1:
                                for _ in range(per_slot):
                                    if pending:
                                        pending.pop(0)()
                        for j in range(max(jmax + 1 - depth, 0), jmax + 1):
                            denav(j)

                        # den was broadcast to all 128 partitions by the ones
                        # matmul, so no cross-partition broadcast is needed
                        rbc = wkp.tile([128, 512], f32, tag="rbc", bufs=2)
                        nc.vector.reciprocal_approx_fast(rbc[:], pden[:])
                        nc.vector.tensor_mul(AT[bi][h][:, qs], pav[:], rbc[:])

                    # drain any outproj tiles that didn't fit in the j-loops
                    for fn in pending:
                        fn()
                # final i4's outproj has no later attention to hide in
                if bi == b - 1:
                    for fn in outproj_tiles(bi, nt - 1, last=True):
                        fn()

    nc.compile()
    return nc


def host_inputs(x, Wq, Wk, Wv, Wp, ncores=NCORES, mmdt="bf16"):
    import ml_dtypes

    mdt = np.float32 if mmdt == "f32r" else ml_dtypes.bfloat16
    """Per-core input dicts (sharding + layout prep on host)."""
    b, t, c = x.shape
    d = D
    xT = np.ascontiguousarray(np.transpose(x, (0, 2, 1)))  # [B, C, T]
    inv = (1.0 / (10000.0 ** (np.arange(0, d, 2, dtype=np.float32) / np.float32(d)))).astype(np.float32)
    pos = np.arange(t, dtype=np.float32)
    fr = np.outer(pos, inv).astype(np.float32)  # [T, 64]
    cosT = np.cos(fr).T.astype(np.float32)  # [64, T]
    sinT = np.sin(fr).T.astype(np.float32)
    # pair-interleaved rope tables: partition 2m,2m+1 <- freq m; sign -/+ on sin
    cosI = np.ascontiguousarray(np.repeat(cosT, 2, axis=0))  # [128, T]
    sinS = np.ascontiguousarray(
        np.stack([-sinT, sinT], axis=1).reshape(128, t)
    )
    # column permutation putting rope pair (m, m+64) at (2m, 2m+1), per head
    perm = np.stack([np.arange(64), np.arange(64) + 64], 1).reshape(128)
    triu = np.triu(np.ones((128, 128), np.float32))
    maskf = np.ascontiguousarray(
        np.concatenate([np.zeros((128, 384), np.float32), triu], 1)
    )
    onesv = np.ones((128, 128), np.float32)

    def permute_heads(w):
        # w: [c, nheads*d] -> same with each head's columns permuted by perm
        nh = w.shape[1] // d
        wv_ = w.reshape(w.shape[0], nh, d)
        return np.ascontiguousarray(wv_[:, :, perm].reshape(w.shape))

    Wq_p = permute_heads(Wq)
    Wk_p = permute_heads(Wk)

    xTm = xT.astype(mdt) if mdt is not np.float32 else xT
    in_maps = []
    for ci in range(ncores):
        qs = slice(ci * HL * d, (ci + 1) * HL * d)
        in_maps.append(
            {
                "xT": xTm,
                "wq": np.ascontiguousarray(Wq_p[:, qs]).astype(mdt),
                "wk": np.ascontiguousarray(Wk_p[:, ci * d : (ci + 1) * d]).astype(mdt),
                "wv": np.ascontiguousarray(Wv[:, ci * d : (ci + 1) * d]).astype(mdt),
                "wp": np.ascontiguousarray(Wp[qs, :]).astype(mdt),
                "cos2": cosI,
                "sin2": sinS,
                "maskf": maskf.astype(mdt),
                "onesv": onesv.astype(mdt),
                "ident": np.eye(128, dtype=np.float32).astype(mdt),
            }
        )
    return in_maps


_NC_CACHE = {}

MMDT = "bf16"


def _get_nc(mmdt=None):
    mmdt = mmdt or MMDT
    key = (B, T, C, mmdt)
    if key not in _NC_CACHE:
        _NC_CACHE[key] = build_nc(B, T, C, mmdt=mmdt)
    return _NC_CACHE[key]


def _install_cc_error_surfacing():
    """Make neuronx_cc hook failures print a real traceback instead of the
    opaque PJRT 'py_result' error."""
    try:
        from concourse import bass2jax

        bass2jax.install_neuronx_cc_hook()
        import libneuronxla

        if getattr(libneuronxla, "_tb_wrapped", False):
            return
        inner = libneuronxla.neuronx_cc

        def wrapped(*a, **k):
            try:
                return inner(*a, **k)
            except BaseException:
                import traceback

                traceback.print_exc()
                raise

        libneuronxla.neuronx_cc = wrapped
        libneuronxla._tb_wrapped = True
    except Exception:
        pass


def run_spmd(x, Wq, Wk, Wv, Wp, trace=False, mmdt=None):
    from concourse.bass_utils import run_bass_kernel_spmd

    mmdt = mmdt or MMDT
    _install_cc_error_surfacing()

    nc = _get_nc(mmdt)
    in_maps = host_inputs(x, Wq, Wk, Wv, Wp, mmdt=mmdt)
    last_err = None
    for attempt in range(3):
        try:
            res = run_bass_kernel_spmd(
                nc, in_maps, core_ids=list(range(NCORES)), trace=trace
            )
            break
        except Exception as e:  # transient NRT device faults: retry
            last_err = e
            import time as _time

            _time.sleep(5.0)
    else:
        raise last_err
    acc = res.results[0]["y"].astype(np.float64)
    for i in range(1, NCORES):
        acc += res.results[i]["y"]
    return acc.astype(np.float32), res


def kernel(x, Wq, Wk, Wv, Wp):
    out, _ = run_spmd(x, Wq, Wk, Wv, Wp, trace=False)
    return out
